# revision 12
# baseline (speedup 1.0000x reference)
"""Trainium2 Bass kernel for MessagePassingLayerEC (gnn_message_passing).

Math (reference):
    src_proj  = node_values @ W_src + b_src            # [V, D]
    dest_proj = node_values @ W_dest + b_dest          # [V, D]
    msgs = relu(src_proj[edge_src] + dest_proj[edge_dest] + edge_emb[edge_cls])
    out  = segment_sum(msgs, edge_dest, V)             # [V, D]

Strategy (8 cores, edge-parallel, dest-contiguous ownership => no all-reduce):
  - Host sorts edges by dest; segments (dests) pack into groups of <= 96
    segments and <= 8 gather tiles (128 edges each).  Edges within a group
    sort by src and split into two windows (src < 25000 / >= 25000) so
    int16 gather indices are offsets into a 32k-row table slice.
  - Per super-batch of 8 groups, all w0 tiles then all w1 tiles; each
    window's slots gather with 2 dma_gathers => 4 gathers on 4 SWDGE
    queues.
  - dest_proj + edge_emb apply via ONE one-hot matmul per 128-edge block:
    combo rows 0..95 = the group's dest rows (resident SBUF slab, written
    directly by the projection pass), rows 96..127 = 32 emb classes with
    biases folded.  No dest gather.
  - Segment one-hots (lhsT of the reduce matmul) build on DVE with one
    broadcast is_equal per 512-edge chunk.
  - Each group owns a disjoint contiguous 128-row range of the output, so
    the result DMAs out as plain contiguous stores (no scatter, no
    zeroing); the host unpack reads only the valid rows per group.
"""

import sys

if "/opt/trn_rl_repo" not in sys.path:
    sys.path.insert(0, "/opt/trn_rl_repo")

import numpy as np
import ml_dtypes

BF16 = ml_dtypes.bfloat16

P = 128
NTILE_G = 8         # gather tiles (128-edge blocks) per group
GSLOT = NTILE_G * P          # edge slots per group (1024)
MAXSEG = 96         # segments per group (combo rows 96..127 = emb)
SB_G = 8            # groups per super-batch
SBT = SB_G * NTILE_G         # tiles per super-batch (64)
NQ = 4              # SWDGE queues
WSPLIT = 24576      # src window boundary (multiple of 2048: table lo/hi split)
NC_CORES = 8

V_GLOBAL = 50000
E_GLOBAL = 640000
DIM = 128
NCLS = 32


def _round_up(x, m):
    return (x + m - 1) // m * m


def _wrap_idx16(flat):
    """dma_gather index layout: idx j -> [j%16, j//16], replicated 8x down
    partitions; packed into int32 pairs for PJRT friendliness."""
    n = flat.shape[0]
    assert n % 32 == 0
    w = np.zeros((P, n // 16), dtype=np.int16)
    blk = flat.reshape(n // 16, 16).T
    for g in range(8):
        w[g * 16:(g + 1) * 16, :] = blk
    return np.ascontiguousarray(w).view(np.int32)


# ---------------------------------------------------------------------------
# Host-side packing
# ---------------------------------------------------------------------------

def _host_pack(node_values, edge_src, edge_dest, edge_cls,
               W_src, b_src, W_dest, b_dest, edge_emb, n_cores=NC_CORES):
    V, D = node_values.shape
    E = edge_src.shape[0]

    order = np.argsort(edge_dest, kind="stable")
    ds_ = edge_dest[order].astype(np.int64)
    ss_ = edge_src[order].astype(np.int64)
    cs_ = edge_cls[order].astype(np.int64)

    first = np.empty(E, dtype=bool)
    first[0] = True
    first[1:] = ds_[1:] != ds_[:-1]
    seg_starts = np.flatnonzero(first)
    nseg = len(seg_starts)
    seg_ends = np.append(seg_starts[1:], E)
    seg_dest = ds_[seg_starts]

    # split segments into n_cores chunks with ~equal edge counts
    seg_cut = [0]
    for k in range(1, n_cores):
        tgt = k * E // n_cores
        i = np.searchsorted(seg_starts, tgt)
        i = min(max(i, 1), nseg - 1)
        seg_cut.append(i)
    seg_cut.append(nseg)

    hi_counts = np.add.reduceat((ss_ >= WSPLIT).astype(np.int64), seg_starts)
    seg_lens = seg_ends - seg_starts

    # greedy group packing per core: <=MAXSEG segs, <=WCAP edges per window
    WCAP = NTILE_G // 2 * P
    core_groups = []
    for k in range(n_cores):
        lo, hi = seg_cut[k], seg_cut[k + 1]
        groups = []
        g_lo = lo
        n0 = n1 = gseg = 0
        for g in range(lo, hi):
            e1 = int(hi_counts[g])
            e0 = int(seg_lens[g]) - e1
            if g > g_lo and (n0 + e0 > WCAP or n1 + e1 > WCAP
                             or gseg + 1 > MAXSEG):
                groups.append((g_lo, g))
                g_lo = g
                n0 = n1 = gseg = 0
            n0 += e0
            n1 += e1
            gseg += 1
        groups.append((g_lo, hi))
        core_groups.append(groups)

    NG = _round_up(max(len(g) for g in core_groups), SB_G)
    NSB = NG // SB_G
    SLAB_COLS = _round_up(NG * P, 2048)
    NG_PAD = SLAB_COLS // P      # slab groups incl. projection-pass padding
    OUT_ROWS = NG * P
    VP = _round_up(V, 2048)

    nodesT = np.zeros((D, VP), dtype=BF16)
    nodesT[:, :V] = np.ascontiguousarray(node_values.T).astype(BF16)

    def _perm_cols(tbl):
        # column (c*512 + j*128 + p) <- node (c*512 + 4p + j): makes each
        # phase-1 output partition hold 4 consecutive rows (1KB descriptors)
        n = tbl.shape[1]
        pos = np.arange(n)
        node = (pos // 512) * 512 + 4 * (pos % 128) + (pos // 128) % 4
        return np.ascontiguousarray(tbl[:, node])

    emb_eff = (edge_emb + b_src[None, :] + b_dest[None, :]).astype(np.float32)
    emb_pad = np.zeros((P, D), dtype=BF16)
    emb_pad[MAXSEG:MAXSEG + NCLS, :] = emb_eff.astype(BF16)

    # iota4[p, blk*128 + j] = j  (f32, for the batched is_equal)
    iota4 = np.tile(np.arange(P, dtype=np.float32), (P, 4)).astype(np.float32)

    nodesT_perm = _perm_cols(nodesT)

    NBLK = NG * NTILE_G          # 128-edge blocks per core
    SBW = SBT * P // 32          # idx int32 cols per sb (256)
    SGW = SBT                    # sgid cols per sb (64)
    MW = SBW + SGW

    in_maps = []
    asm = []
    for k in range(n_cores):
        groups = core_groups[k]

        idx_flat = np.zeros(NG * GSLOT, dtype=np.int16)
        sgid = np.full((P, NBLK), 127.0, dtype=np.float32)
        oht = np.zeros((P, NBLK * P), dtype=BF16)
        slab_nodes = np.zeros(SLAB_COLS, dtype=np.int64)
        out_rows_l = []
        out_dest_l = []

        for gi, (glo, ghi) in enumerate(groups):
            nsg = ghi - glo
            assert nsg <= MAXSEG
            slab_nodes[gi * P:gi * P + nsg] = seg_dest[glo:ghi]
            out_rows_l.append(gi * P + np.arange(nsg))
            out_dest_l.append(seg_dest[glo:ghi])

            e_idx = np.concatenate([
                np.arange(int(seg_starts[s]), int(seg_ends[s]))
                for s in range(glo, ghi)])
            e_seg = np.concatenate([
                np.full(int(seg_ends[s] - seg_starts[s]), s - glo)
                for s in range(glo, ghi)])
            src = ss_[e_idx]
            o = np.argsort(src, kind="stable")
            e_idx, e_seg, src = e_idx[o], e_seg[o], src[o]
            sb, g = gi // SB_G, gi % SB_G
            for w in range(2):
                m = (src >= WSPLIT) == (w == 1)
                ei, es, sr = e_idx[m], e_seg[m], src[m]
                n = len(ei)
                assert n <= WCAP, (gi, w, n)
                # w0 tiles of group g at sb-blocks [g*4, ..); w1 at 32 +
                i = np.arange(n)
                blk = (sb * SBT + w * SBT // 2 + g * (NTILE_G // 2)
                       + i // P)
                pp = i % P
                idx_flat[blk * P + pp] = sr - WSPLIT * w
                sgid[pp, blk] = es
                oht[es, blk * P + pp] = 1.0
                oht[MAXSEG + cs_[ei], blk * P + pp] = 1.0

        nodesT_slab = nodesT[:, slab_nodes]

        meta = np.zeros((P, NSB * MW), dtype=np.int32)
        for sb in range(NSB):
            c0 = sb * MW
            meta[:, c0:c0 + SBW] = _wrap_idx16(
                idx_flat[sb * SBT * P:(sb + 1) * SBT * P])
            meta[:, c0 + SBW:c0 + MW] = \
                sgid[:, sb * SBT:(sb + 1) * SBT].view(np.int32)

        in_maps.append({
            "nodesT": nodesT_perm,
            "nodesT_slab": np.ascontiguousarray(nodesT_slab),
            "W_src": np.ascontiguousarray(W_src).astype(BF16),
            "W_dest": np.ascontiguousarray(W_dest).astype(BF16),
            "emb_pad": emb_pad,
            "iota4": iota4,
            "meta": meta,
            "onehotT": oht,
        })
        asm.append((np.concatenate(out_rows_l), np.concatenate(out_dest_l)))

    params = dict(NG=int(NG), SLAB_COLS=int(SLAB_COLS), NG_PAD=int(NG_PAD),
                  OUT_ROWS=int(OUT_ROWS), VP=int(VP), D=int(D))
    return in_maps, asm, params


# ---------------------------------------------------------------------------
# Bass kernel
# ---------------------------------------------------------------------------

def build_kernel(params):
    import concourse.bass as bass
    import concourse.mybir as mybir
    import concourse.tile as tile
    from concourse import bacc

    NG = params["NG"]
    SLAB_COLS = params["SLAB_COLS"]
    NG_PAD = params["NG_PAD"]
    OUT_ROWS = params["OUT_ROWS"]
    VP = params["VP"]
    D = params["D"]
    NSB = NG // SB_G
    NBLK = NG * NTILE_G
    SBW = SBT * P // 32
    SGW = SBT
    MW = SBW + SGW

    f32 = mybir.dt.float32
    bf16 = mybir.dt.bfloat16
    i32 = mybir.dt.int32
    i16 = mybir.dt.int16

    nc = bacc.Bacc("TRN2", target_bir_lowering=False, num_swdge_queues=NQ)

    nodesT = nc.dram_tensor("nodesT", [D, VP], bf16, kind="ExternalInput")
    nodesT_slab = nc.dram_tensor("nodesT_slab", [D, SLAB_COLS], bf16,
                                 kind="ExternalInput")
    W_src = nc.dram_tensor("W_src", [D, D], bf16, kind="ExternalInput")
    W_dest = nc.dram_tensor("W_dest", [D, D], bf16, kind="ExternalInput")
    emb_pad = nc.dram_tensor("emb_pad", [P, D], bf16, kind="ExternalInput")
    iota4_d = nc.dram_tensor("iota4", [P, 4 * P], f32, kind="ExternalInput")
    meta = nc.dram_tensor("meta", [P, NSB * MW], i32, kind="ExternalInput")
    onehotT = nc.dram_tensor("onehotT", [P, NBLK * P], bf16,
                             kind="ExternalInput")

    # src projection table split at WSPLIT so the low-window gathers can
    # start as soon as the low half of the projection is written
    VHI = VP - WSPLIT
    src_lo = nc.dram_tensor("src_lo", [WSPLIT, D], bf16, kind="Internal")
    src_hi = nc.dram_tensor("src_hi", [VHI, D], bf16, kind="Internal")
    out = nc.dram_tensor("out", [OUT_ROWS, D], f32, kind="ExternalOutput")

    with tile.TileContext(nc) as tc, tc.tile_pool(name="const", bufs=1) as cpool:
        w_src_sb = cpool.tile([D, D], bf16, tag="wsrc")
        nc.sync.dma_start(w_src_sb[:], W_src[:, :])
        w_dest_sb = cpool.tile([D, D], bf16, tag="wdest")
        nc.sync.dma_start(w_dest_sb[:], W_dest[:, :])
        emb_sb = cpool.tile([P, D], bf16, tag="embp")
        nc.sync.dma_start(emb_sb[:], emb_pad[:, :])
        iota4_sb = cpool.tile([P, 4, P], f32, tag="iota4")
        nc.sync.dma_start(iota4_sb[:], iota4_d[:, :].rearrange(
            "p (c j) -> p c j", c=4))
        slab_sb = cpool.tile([P, NG_PAD, D], bf16, tag="slab")

        # all meta tiles up front: the gathers need them, and anything
        # issued later queues behind phase 1 on the DMA rings
        meta_sb = cpool.tile([P, NSB, MW], i32, tag="meta")
        nc.sync.dma_start(meta_sb[:], meta[:, :].rearrange(
            "p (s w) -> p s w", s=NSB))

        # emb rows 96..127 of every group's slab chunk (const, no dep);
        # on DVE so gpsimd stays free for gather descriptor generation
        for g in range(NG):
            nc.vector.tensor_copy(slab_sb[MAXSEG:P, g, :],
                                  emb_sb[MAXSEG:P, :])

        # ---------------- phases (single pool scope) ----------------
        with (
            tc.tile_pool(name="p1", bufs=3) as p1pool,
            tc.tile_pool(name="p1ps", bufs=2, space="PSUM") as p1ps,
            tc.tile_pool(name="oht", bufs=3) as opool,
            tc.tile_pool(name="gath", bufs=4) as gpool,
            tc.tile_pool(name="work", bufs=4) as wpool,
            tc.tile_pool(name="msgs", bufs=3) as mspool,
            tc.tile_pool(name="segout", bufs=3) as spool,
            tc.tile_pool(name="psmsg", bufs=3, space="PSUM") as psmsg,
            tc.tile_pool(name="psseg", bufs=2, space="PSUM") as psseg,
        ):
            def load_oht(sb):
                # 4 chunked DMAs (4KB per partition each) so gather rows
                # interleave with these on the shared DMA engines
                t = opool.tile([P, SBT * P], bf16, tag="oht")
                quarter = SBT * P // 4
                for k in range(4):
                    nc.sync.dma_start(
                        t[:, k * quarter:(k + 1) * quarter],
                        onehotT[:, sb * SBT * P + k * quarter:
                                sb * SBT * P + (k + 1) * quarter])
                return t

            # dest slab: straight into resident SBUF (no DRAM round-trip)
            nsup_d = SLAB_COLS // 2048
            for su in range(nsup_d):
                nt_sb = p1pool.tile([D, 2048], bf16, tag="p1in")
                nc.sync.dma_start(
                    nt_sb[:], nodesT_slab[:, su * 2048:(su + 1) * 2048])
                for cc in range(4):
                    ps = p1ps.tile([P, 512], f32, tag="p1ps")
                    for j in range(4):
                        nc.tensor.matmul(
                            ps[:, j * P:(j + 1) * P],
                            lhsT=nt_sb[:, cc * 512 + j * P:
                                       cc * 512 + (j + 1) * P],
                            rhs=w_dest_sb[:],
                            start=True, stop=True,
                        )
                    g0 = su * 16 + cc * 4
                    nc.scalar.activation(
                        slab_sb[0:MAXSEG, g0:g0 + 4, :], ps[0:MAXSEG, :],
                        mybir.ActivationFunctionType.Copy)

            # src table -> DRAM (permuted for 1KB gather descriptors);
            # low half first so the w0 gathers can start early
            oht_pre = {}
            for half, dram in ((0, src_lo), (1, src_hi)):
                base = 0 if half == 0 else WSPLIT
                ncols = WSPLIT if half == 0 else VP - WSPLIT
                dview = dram[:, :].rearrange("(c p r) d -> p c (r d)",
                                             p=P, r=4)
                for su in range(ncols // 2048):
                    c0 = base + su * 2048
                    nt_sb = p1pool.tile([D, 2048], bf16, tag="p1in")
                    nc.sync.dma_start(nt_sb[:], nodesT[:, c0:c0 + 2048])
                    ob = p1pool.tile([P, 4, 512], bf16, tag="p1out")
                    for cc in range(4):
                        ps = p1ps.tile([P, 512], f32, tag="p1ps")
                        for j in range(4):
                            nc.tensor.matmul(
                                ps[:, j * P:(j + 1) * P],
                                lhsT=nt_sb[:, cc * 512 + j * P:
                                           cc * 512 + (j + 1) * P],
                                rhs=w_src_sb[:],
                                start=True, stop=True,
                            )
                        nc.scalar.activation(
                            ob[:, cc, :], ps[:],
                            mybir.ActivationFunctionType.Copy)
                    nc.sync.dma_start(
                        dview[:, su * 4:(su + 1) * 4, :], ob[:])
                if half == 0:
                    # first ohts ride the rings while the w0 gathers run
                    for sb in range(min(3, NSB)):
                        oht_pre[sb] = load_oht(sb)

            # ---------------- phase 2: edges ----------------
            for sb in range(NSB):
                ia = meta_sb[:, sb, 0:SBW]
                sgid = meta_sb[:, sb, SBW:MW].bitcast(f32)
                oht = oht_pre.pop(sb) if sb in oht_pre else load_oht(sb)

                ga = gpool.tile([P, SBT, D], bf16, tag="ga")
                # 4 sub-gathers: windows (w0 tiles | w1 tiles), each halved
                for q in range(NQ):
                    t0c = q * (SBT // NQ)
                    t1c = t0c + SBT // NQ
                    tbl = src_lo if q < 2 else src_hi
                    nidx = (t1c - t0c) * P
                    nc.gpsimd.dma_gather(
                        ga[:, t0c:t1c, :],
                        tbl[:, :],
                        ia[:, t0c * 4:t1c * 4].bitcast(i16),
                        nidx, nidx, D,
                        single_packet=False, queue_num=q)

                # chunk c covers blocks [4c, 4c+4) -> all of group c % 8
                # (w0 chunks 0..7, w1 chunks 8..15); process each group's
                # two chunks back-to-back so only one PSUM segment
                # accumulation window is open at a time.
                seg_sb = spool.tile([P, SB_G, D], f32, tag="segsb")
                for gl in range(SB_G):
                    ps_seg = psseg.tile([P, P], f32, tag="psseg")
                    for c in (gl, gl + SB_G):
                        ps_m = psmsg.tile([P, 512], f32, tag="psmsg")
                        for j in range(4):
                            blk = c * 4 + j
                            nc.tensor.matmul(
                                ps_m[:, j * P:(j + 1) * P],
                                lhsT=oht[:, blk * P:(blk + 1) * P],
                                rhs=slab_sb[:, sb * SB_G + gl, :],
                                start=True, stop=True,
                            )
                        # gt4[p, j, s] = (sgid[p, blk] == s): one broadcast
                        # is_equal for all 4 blocks of the chunk
                        gt4 = wpool.tile([P, 4, P], bf16, tag="gt4")
                        nc.vector.tensor_tensor(
                            out=gt4[:],
                            in0=sgid[:, c * 4:(c + 1) * 4, None].broadcast_to(
                                [P, 4, P]),
                            in1=iota4_sb[:],
                            op=mybir.AluOpType.is_equal)
                        t3 = wpool.tile([P, 512], f32, tag="t3")
                        nc.vector.tensor_tensor(
                            out=t3[:],
                            in0=ga[:, c * 4:(c + 1) * 4, :].rearrange(
                                "p t e -> p (t e)"),
                            in1=ps_m[:],
                            op=mybir.AluOpType.add)
                        msgs = mspool.tile([P, 512], bf16, tag="msgs")
                        nc.scalar.activation(
                            msgs[:], t3[:],
                            mybir.ActivationFunctionType.Relu)
                        for j in range(4):
                            nc.tensor.matmul(
                                ps_seg[:],
                                lhsT=gt4[:, j, :],
                                rhs=msgs[:, j * P:(j + 1) * P],
                                start=(c == gl and j == 0),
                                stop=(c == gl + SB_G and j == 3))
                    nc.scalar.activation(seg_sb[:, gl, :], ps_seg[:],
                                         mybir.ActivationFunctionType.Copy)
                nc.sync.dma_start(
                    out[:, :].rearrange("(g p) d -> p g d", p=P)[
                        :, sb * SB_G:(sb + 1) * SB_G, :],
                    seg_sb[:])

    nc.compile()
    return nc


# ---------------------------------------------------------------------------
# Entry point
# ---------------------------------------------------------------------------

def kernel(**inputs):
    node_values = np.asarray(inputs["node_values"], dtype=np.float32)
    edge_src = np.asarray(inputs["edge_src"], dtype=np.int32)
    edge_dest = np.asarray(inputs["edge_dest"], dtype=np.int32)
    edge_cls = np.asarray(inputs["edge_cls"], dtype=np.int32)
    W_src = np.asarray(inputs["W_src"], dtype=np.float32)
    b_src = np.asarray(inputs["b_src"], dtype=np.float32)
    W_dest = np.asarray(inputs["W_dest"], dtype=np.float32)
    b_dest = np.asarray(inputs["b_dest"], dtype=np.float32)
    edge_emb = np.asarray(inputs["edge_emb"], dtype=np.float32)

    V = node_values.shape[0]

    in_maps, asm, params = _host_pack(
        node_values, edge_src, edge_dest, edge_cls,
        W_src, b_src, W_dest, b_dest, edge_emb)

    nc = build_kernel(params)

    from concourse.bass_utils import run_bass_kernel_spmd
    res = run_bass_kernel_spmd(nc, in_maps, core_ids=list(range(NC_CORES)))

    out = np.zeros((V, DIM), dtype=np.float32)
    for k in range(NC_CORES):
        rows, dests = asm[k]
        out[dests] = np.asarray(res.results[k]["out"])[rows]
    return out


if __name__ == "__main__":
    rng = np.random.default_rng(0)
    V, E = V_GLOBAL, E_GLOBAL
    ins = {
        "node_values": rng.normal(size=(V, DIM)).astype(np.float32),
        "edge_src": rng.integers(0, V, size=E).astype(np.int32),
        "edge_dest": rng.integers(0, V, size=E).astype(np.int32),
        "edge_cls": rng.integers(0, NCLS, size=E).astype(np.int32),
        "W_src": (rng.normal(size=(DIM, DIM)) / np.sqrt(DIM)).astype(np.float32),
        "b_src": np.zeros(DIM, dtype=np.float32),
        "W_dest": (rng.normal(size=(DIM, DIM)) / np.sqrt(DIM)).astype(np.float32),
        "b_dest": np.zeros(DIM, dtype=np.float32),
        "edge_emb": rng.normal(size=(NCLS, DIM)).astype(np.float32),
    }
    out = kernel(**ins)
    print("out", out.shape, out.dtype, float(np.abs(out).sum()))
